# revision 30
# baseline (speedup 1.0000x reference)
"""MoE-routed per-node linear (ElementLinear) on 8 TRN2 NeuronCores.

Math (per node b): out[b] = (weights[argmax(node_attrs[b])] * ALPHA) @ T[b] + bias[argmax(...)][:, None]
  T: [B=20000, C=128, R=16] f32, node_attrs: [B, E=10], weights: [E, OUT=128, C], bias: [E, OUT]

Strategy: data-parallel over B (2500 nodes/core).  On device per core:
  1. argmax over the 10 experts (vector engine, [16, W, 10] wrapped layout)
  2. per expert: sparse_gather compacts the node-id list (the permutation),
     padded with -1 to a static capacity KCAP
  3. dma_gather(transpose=True) pulls that expert's node blocks from HBM
     directly into [c=128 partitions, r, slot] bf16 layout (zero traffic for
     the -1 tail via num_idxs_reg loaded from sparse_gather's num_found)
  4. token-stationary matmuls: lhsT = X chunk (slots), rhs = W[e]^T -> psum
     [slot, out]; bias added via a K=1 matmul of ones^T @ bias_row
  5. psum -> sbuf staging (bf16, slot-major rows) -> dma_scatter_add back to
     the pre-zeroed output (rows written exactly once; trailing -1 skipped)

Host side only stages layout/dtype (transpose + bf16 cast + constant tables).
"""

import os
import sys

import numpy as np

for _p in ("/opt/trn_rl_repo", "/root/.axon_site/_ro/trn_rl_repo"):
    if os.path.isdir(_p) and _p not in sys.path:
        sys.path.insert(0, _p)

import ml_dtypes  # noqa: E402

import concourse.bacc as bacc  # noqa: E402
import concourse.bass as bass  # noqa: E402
import concourse.mybir as mybir  # noqa: E402
import concourse.tile as tile  # noqa: E402
from concourse.bass_utils import run_bass_kernel_spmd  # noqa: E402

BF16 = ml_dtypes.bfloat16

# Problem constants (hardcoded per spec).
B, C, R, E, OUT = 20000, 128, 16, 10, 128
NCORES = 8
ALPHA = float(1.0 / np.sqrt(np.float32(C)).astype(np.float32))
BIG = 1024.0  # > E; sentinel offset for the argmin-of-max trick
CR = C * R  # 2048 elements per node block


class Cfg:
    def __init__(self, bc=B // NCORES, w16=None, kcap=384):
        self.bc = bc                      # real nodes per core
        self.w16 = w16 if w16 is not None else -(-bc // 16)
        self.bcp = 16 * self.w16          # padded nodes per core
        self.kcap = kcap                  # per-expert slot capacity (mult of 128)
        assert self.kcap % 128 == 0
        assert self.kcap % 16 == 0
        assert E * self.kcap >= self.bcp


FULL = Cfg()


def build_nc(cfg=FULL, debug=False, dump=False, stage_limit=5, host_idx=False):
    """stage_limit: 1=argmax only, 2=+sparse/idx, 3=+gather, 4=+matmul, 5=full.
    host_idx: take per-expert indices/counts from host params (debug)."""
    """Build the single-core Bass graph (SPMD: same graph on all 8 cores)."""
    fp32 = mybir.dt.float32
    bf16 = mybir.dt.bfloat16
    i16 = mybir.dt.int16
    u32 = mybir.dt.uint32

    W = cfg.w16
    KC = cfg.kcap
    KCOLS = KC // 16          # compacted index columns per expert
    NJ = KC // 128            # 128-slot blocks per expert
    AluOp = mybir.AluOpType

    nc = bacc.Bacc("TRN2", target_bir_lowering=False, debug=debug,
                   num_devices=NCORES)

    t_in = nc.declare_dram_parameter("t_in", [cfg.bcp, CR], bf16, isOutput=False)
    attrs_in = nc.declare_dram_parameter("attrs", [16, W * E], fp32, isOutput=False)
    wt_in = nc.declare_dram_parameter("wt", [C, E * OUT], fp32, isOutput=False)
    bias4_in = nc.declare_dram_parameter("bias4", [1, E * 512], fp32, isOutput=False)
    iota_in = nc.declare_dram_parameter("iota_c", [16, W * E], fp32, isOutput=False)
    bp1_in = nc.declare_dram_parameter("bp1_c", [16, W], fp32, isOutput=False)
    seq_in = nc.declare_dram_parameter("seq_c", [16, KCOLS], fp32, isOutput=False)
    out_ext = nc.declare_dram_parameter("out", [cfg.bcp, CR], bf16, isOutput=True)
    if host_idx:
        hidx_in = nc.declare_dram_parameter("hidx", [E * 128, cfg.kcap // 16], i16, isOutput=False)
        hnf_in = nc.declare_dram_parameter("hnf", [1, E], u32, isOutput=False)
    if dump:
        idxw_dbg = nc.declare_dram_parameter("idxw_dbg", [16, cfg.w16], fp32, isOutput=True)
        idx128_dbg = nc.declare_dram_parameter("idx128_dbg", [128, cfg.kcap // 16], i16, isOutput=True)
        xg_dbg = nc.declare_dram_parameter("xg_dbg", [128, R * cfg.kcap], bf16, isOutput=True)
        stage_dbg = nc.declare_dram_parameter("stage_dbg", [128, (cfg.kcap // 128) * CR], bf16, isOutput=True)
        comp_dbg = nc.declare_dram_parameter("comp_dbg", [16, E * (cfg.kcap // 16)], fp32, isOutput=True)
        nf_dbg = nc.declare_dram_parameter("nf_dbg", [1, E], u32, isOutput=True)

    # one gpsimd register per expert for the runtime valid-index count
    nregs = [nc.alloc_register(mybir.EngineType.Pool, f"nf{e}") for e in range(E)]

    with tile.TileContext(nc) as tc:
        with (
            tc.tile_pool(name="const", bufs=1) as cpool,
            tc.tile_pool(name="route", bufs=2) as rpool,
            tc.tile_pool(name="xg", bufs=3) as xpool,
            tc.tile_pool(name="stg", bufs=2) as spool,
            tc.tile_pool(name="ps", bufs=6, space="PSUM") as ppool,
            tc.tile_pool(name="psr", bufs=2, space="PSUM") as prpool,
        ):
            # ---- weights / bias prep ----
            wt_f32 = cpool.tile([C, E * OUT], fp32)
            nc.sync.dma_start(out=wt_f32[:], in_=wt_in[:])
            wt_sb = cpool.tile([C, E * OUT], bf16)
            nc.vector.tensor_scalar_mul(wt_sb[:], wt_f32[:], ALPHA)

            bias_f32 = cpool.tile([1, E * 512], fp32)
            nc.sync.dma_start(out=bias_f32[:], in_=bias4_in[:])
            bias_sb = cpool.tile([1, E * 512], bf16)
            nc.vector.tensor_copy(bias_sb[:], bias_f32[:])

            ones1 = cpool.tile([1, OUT], bf16)
            nc.vector.memset(ones1[:], 1.0)

            ones16 = cpool.tile([1, 16], fp32)
            nc.vector.memset(ones16[:], 1.0)
            seqpos = cpool.tile([16, KCOLS], fp32)
            nc.sync.dma_start(out=seqpos[:], in_=seq_in[:])
            neg1 = cpool.tile([16, KCOLS], fp32)
            nc.vector.memset(neg1[:], -1.0)

            # ---- argmax routing ([16, W, E] wrapped; node b = q*W + w) ----
            attrs_sb = rpool.tile([16, W * E], fp32, tag="attrs")
            nc.sync.dma_start(out=attrs_sb[:], in_=attrs_in[:])
            iota_sb = cpool.tile([16, W * E], fp32)
            nc.sync.dma_start(out=iota_sb[:], in_=iota_in[:])
            bp1_sb = cpool.tile([16, W], fp32)
            nc.sync.dma_start(out=bp1_sb[:], in_=bp1_in[:])

            a3 = attrs_sb[:].rearrange("p (w j) -> p w j", j=E)
            mx = rpool.tile([16, W], fp32, tag="mx")
            nc.vector.tensor_reduce(out=mx[:], in_=a3, op=AluOp.max,
                                    axis=mybir.AxisListType.X)
            onehot = rpool.tile([16, W * E], fp32, tag="onehot")
            nc.vector.tensor_tensor(
                out=onehot[:].rearrange("p (w j) -> p w j", j=E),
                in0=a3,
                in1=mx[:, :, None].to_broadcast([16, W, E]),
                op=AluOp.is_equal,
            )
            # cand = onehot * (iota - BIG); reduce_min -> argmax_j - BIG
            cand = rpool.tile([16, W * E], fp32, tag="cand")
            nc.vector.tensor_tensor(out=cand[:], in0=onehot[:], in1=iota_sb[:],
                                    op=AluOp.mult)
            idxw = rpool.tile([16, W], fp32, tag="idxw")
            nc.vector.tensor_reduce(out=idxw[:],
                                    in_=cand[:].rearrange("p (w j) -> p w j", j=E),
                                    op=AluOp.min, axis=mybir.AxisListType.X)
            nc.vector.tensor_scalar_add(idxw[:], idxw[:], BIG)
            if dump:
                nc.sync.dma_start(out=idxw_dbg[:], in_=idxw[:])

            # ---- per-expert pipeline ----
            for e in range(E if stage_limit >= 2 else 0):
                # arr = (idx == e) ? node_id : -1
                mask = rpool.tile([16, W], fp32, tag="mask")
                nc.vector.tensor_scalar(mask[:], idxw[:], float(e), None,
                                        op0=AluOp.is_equal)
                arr = rpool.tile([16, W], fp32, tag="arr")
                nc.vector.tensor_tensor(out=arr[:], in0=mask[:], in1=bp1_sb[:],
                                        op=AluOp.mult)
                nc.vector.tensor_scalar_add(arr[:], arr[:], -1.0)

                compact = rpool.tile([16, KCOLS], fp32, tag="compact")
                nfound = rpool.tile([1, 1], u32, tag="nfound")
                nc.gpsimd.sparse_gather(out=compact[:], in_=arr[:],
                                        num_found=nfound[:])
                if dump:
                    nc.sync.dma_start(out=comp_dbg[:, e * KCOLS:(e + 1) * KCOLS],
                                      in_=compact[:])
                    nc.sync.dma_start(out=nf_dbg[:1, e:e + 1], in_=nfound[:])

                if host_idx:
                    idx128 = rpool.tile([128, KCOLS], i16, tag="idx128")
                    nc.gpsimd.dma_start(out=idx128[:],
                                        in_=hidx_in[e * 128:(e + 1) * 128, :])
                    hnf = rpool.tile([1, 1], u32, tag="hnf")
                    nc.gpsimd.dma_start(out=hnf[:], in_=hnf_in[:1, e:e + 1])
                    nc.gpsimd.reg_load(nregs[e], hnf[:1, :1])
                else:
                    nc.gpsimd.reg_load(nregs[e], nfound[:1, :1])
                    # HW sparse_gather leaves garbage after the num_found
                    # prefix (CoreSim pads -1); the Q7 gather/scatter trim
                    # trailing work by reading idx values, so the tail MUST
                    # be -1.  Mask positions >= nf: idx = m*(v+1) - 1.
                    nff = rpool.tile([1, 1], fp32, tag="nff")
                    nc.vector.tensor_copy(nff[:], nfound[:])
                    nfps = prpool.tile([16, 1], fp32, tag="nfps", space="PSUM")
                    nc.tensor.matmul(nfps[:], lhsT=ones16[:], rhs=nff[:],
                                     start=True, stop=True)
                    nf16 = rpool.tile([16, 1], fp32, tag="nf16")
                    nc.vector.tensor_copy(nf16[:], nfps[:])
                    vmask = rpool.tile([16, KCOLS], mybir.dt.uint8, tag="vmask")
                    nc.vector.tensor_scalar(vmask[:], seqpos[:], nf16[:, :1], None,
                                            op0=AluOp.is_lt)
                    idxc = rpool.tile([16, KCOLS], fp32, tag="idxc")
                    nc.vector.select(idxc[:], vmask[:], compact[:], neg1[:])
                    idx16 = rpool.tile([16, KCOLS], i16, tag="idx16")
                    nc.vector.tensor_copy(idx16[:], idxc[:])
                    # replicate rows 0-15 to all 8 16-partition groups
                    # (one per Q7 core) via SBUF DMA
                    idx128 = rpool.tile([128, KCOLS], i16, tag="idx128")
                    for rep in range(8):
                        nc.sync.dma_start(out=idx128[rep * 16:(rep + 1) * 16, :],
                                          in_=idx16[:])

                if stage_limit < 3:
                    continue

                # gather this expert's node blocks: [c=128, r, slot] bf16
                xg = xpool.tile([128, R, KC], bf16, tag="xg")
                nc.gpsimd.dma_gather(
                    xg[:],
                    t_in[:, :],
                    idx128[:],
                    num_idxs=KC,
                    num_idxs_reg=nregs[e],
                    elem_size=CR,
                    transpose=True,
                )

                if dump and e == 0:
                    nc.sync.dma_start(out=idx128_dbg[:], in_=idx128[:])
                    nc.sync.dma_start(out=xg_dbg[:], in_=xg[:].rearrange("p a b -> p (a b)"))
                if stage_limit < 4:
                    continue

                # matmuls: lhsT = X chunk [c, 128 slots], rhs = W[e]^T [c, out]
                stage = spool.tile([128, NJ, CR], bf16, tag="stage")
                for j in range(NJ):
                    for rg in range(R // 4):
                        ps = ppool.tile([128, 512], fp32, tag="ps", space="PSUM")
                        nc.tensor.matmul(
                            ps[:],
                            lhsT=ones1[:],
                            rhs=bias_sb[:1, e * 512:(e + 1) * 512],
                            start=True, stop=False,
                        )
                        for ri in range(4):
                            r = rg * 4 + ri
                            nc.tensor.matmul(
                                ps[:, ri * 128:(ri + 1) * 128],
                                lhsT=xg[:, r, j * 128:(j + 1) * 128],
                                rhs=wt_sb[:, e * OUT:(e + 1) * OUT],
                                start=False, stop=(ri == 3),
                            )
                        # psum -> staging (alternate DVE / ACT)
                        dst = stage[:, j, rg * 512:(rg + 1) * 512]
                        if (j * (R // 4) + rg) % 2 == 0:
                            nc.vector.tensor_copy(dst, ps[:])
                        else:
                            nc.scalar.copy(dst, ps[:])

                if dump and e == 0:
                    nc.sync.dma_start(out=stage_dbg[:], in_=stage[:].rearrange("p a b -> p (a b)"))
                if stage_limit < 5:
                    continue

                # scatter rows back (out starts zeroed; += writes once per row)
                nc.gpsimd.dma_scatter_add(
                    out_ext[:, :],
                    stage[:],
                    idx128[:],
                    num_idxs=KC,
                    num_idxs_reg=nregs[e],
                    elem_size=CR,
                )

    nc.compile()
    return nc


# ---------------------------------------------------------------------------
# Host-side staging
# ---------------------------------------------------------------------------

def make_constants(cfg=FULL):
    W = cfg.w16
    iota_c = np.tile(np.arange(E, dtype=np.float32) - np.float32(BIG),
                     (16, W, 1)).reshape(16, W * E)
    bp1_c = (np.arange(cfg.bcp, dtype=np.float32) + 1.0).reshape(16, W)
    kcols = cfg.kcap // 16
    seq_c = (np.arange(cfg.kcap, dtype=np.float32)
             .reshape(kcols, 16).T.copy())  # seq position of wrap cell [q, s]
    return iota_c, bp1_c, seq_c


def stage_core_inputs(Tc, attrs_c, weights, bias, cfg=FULL):
    """Build the device input map for one core from its f32 shard."""
    bc = Tc.shape[0]
    W = cfg.w16

    t_st = np.zeros((cfg.bcp, CR), dtype=BF16)
    # node row layout: f = r*128 + c  (so gather-transpose lands c on partitions)
    t_st[:bc] = Tc.transpose(0, 2, 1).reshape(bc, CR).astype(BF16)

    at = np.zeros((cfg.bcp, E), dtype=np.float32)
    at[:bc] = attrs_c
    attrs_w = at.reshape(16, W * E)  # node b = q*W + w at [q, w*E:(w+1)*E]

    wt = np.ascontiguousarray(
        weights.transpose(2, 0, 1).reshape(C, E * OUT)).astype(np.float32)
    bias4 = np.tile(bias[:, None, :], (1, 4, 1)).reshape(1, E * 512).astype(np.float32)

    iota_c, bp1_c, seq_c = make_constants(cfg)
    return {
        "t_in": t_st,
        "attrs": attrs_w,
        "wt": wt,
        "bias4": bias4,
        "iota_c": iota_c,
        "bp1_c": bp1_c,
        "seq_c": seq_c,
    }


def unstage_core_output(out_st, bc, cfg=FULL):
    """Device output [bcp, CR] bf16 (row = r-major) -> [bc, OUT, R] f32."""
    o = np.asarray(out_st[:bc], dtype=np.float32)
    return o.reshape(bc, R, OUT).transpose(0, 2, 1)


_NC_CACHE = {}


def _get_nc(cfg=FULL):
    key = (cfg.bcp, cfg.kcap)
    if key not in _NC_CACHE:
        _NC_CACHE[key] = build_nc(cfg)
    return _NC_CACHE[key]


def kernel(T, node_attrs, weights, bias):
    T = np.asarray(T, dtype=np.float32)
    node_attrs = np.asarray(node_attrs, dtype=np.float32)
    weights = np.asarray(weights, dtype=np.float32)
    bias = np.asarray(bias, dtype=np.float32)

    cfg = FULL
    bc = B // NCORES
    in_maps = []
    for c in range(NCORES):
        sl = slice(c * bc, (c + 1) * bc)
        in_maps.append(stage_core_inputs(T[sl], node_attrs[sl], weights, bias, cfg))

    nc = _get_nc(cfg)
    res = run_bass_kernel_spmd(nc, in_maps, core_ids=list(range(NCORES)))

    outs = [unstage_core_output(res.results[c]["out"], bc, cfg)
            for c in range(NCORES)]
    return np.concatenate(outs, axis=0).reshape(B, OUT, R)


# revision 59
# speedup vs baseline: 1.4237x; 1.4237x over previous
"""MoE-routed per-node linear (ElementLinear) on 8 TRN2 NeuronCores.

Math (per node b): out[b] = (weights[argmax(node_attrs[b])] * ALPHA) @ T[b] + bias[argmax(...)][:, None]
  T: [B=20000, C=128, R=16] f32, node_attrs: [B, E=10], weights: [E, OUT=128, C], bias: [E, OUT]

Strategy: data-parallel over B (2500 nodes/core).  On device per core:
  1. argmax over the 10 experts on [16, W*E] tiles (vector engine)
  2. per expert: arr = node_id if routed to e else -1; gpsimd sparse_gather
     compacts the list; positions >= num_found are forced to -1 (HW leaves
     garbage there, and the Q7 gather/scatter trim trailing work by reading
     the idx values) via an is_lt mask + select keyed off num_found
  3. dma_gather(transpose=True) pulls each expert's node blocks from HBM
     directly into [c=128 partitions, r, slot] bf16 layout (num_idxs_reg
     carries the runtime count loaded from num_found)
  4. token-stationary matmuls: lhsT = X chunk (slots on psum partitions),
     rhs = W[e]^T [c, out] -> psum [slot, out]
  5. psum + broadcast bias table -> bf16 staging (DVE/ACT tensor_tensor) ->
     dma_scatter_add back to the pre-zeroed output (rows written exactly
     once; trailing -1 slots are skipped)

Host side only stages layout/dtype (transpose + bf16 cast + constant tables).
"""

import os
import sys

import numpy as np

for _p in ("/opt/trn_rl_repo", "/root/.axon_site/_ro/trn_rl_repo"):
    if os.path.isdir(_p) and _p not in sys.path:
        sys.path.insert(0, _p)

import ml_dtypes  # noqa: E402

import concourse.bacc as bacc  # noqa: E402
import concourse.bass as bass  # noqa: E402
import concourse.mybir as mybir  # noqa: E402
import concourse.tile as tile  # noqa: E402
from concourse.tile import add_dep_helper  # noqa: E402
from concourse.bass_utils import run_bass_kernel_spmd  # noqa: E402

BF16 = ml_dtypes.bfloat16

# Problem constants (hardcoded per spec).
B, C, R, E, OUT = 20000, 128, 16, 10, 128
NCORES = 8
ALPHA = float(1.0 / np.sqrt(np.float32(C)).astype(np.float32))
BIG = 1024.0  # > E; offset for the argmin-of-max trick
CR = C * R  # 2048 elements per node block


class Cfg:
    def __init__(self, bc=B // NCORES, kcap=384):
        self.bc = bc                       # real nodes per core
        self.w128 = -(-bc // 128)          # node cols in [128, .] layout
        self.bcp = 128 * self.w128         # padded nodes per core
        self.w16 = self.bcp // 16          # node cols in [16, .] layout
        self.kcap = kcap                   # per-expert slot capacity
        self.dummy = self.bcp              # trash row id (gather src / scatter dst)
        assert self.kcap % 128 == 0
        assert E * self.kcap >= self.bcp


FULL = Cfg()


def build_nc(cfg=FULL, debug=False, dump=False):
    fp32 = mybir.dt.float32
    bf16 = mybir.dt.bfloat16
    i16 = mybir.dt.int16
    u32 = mybir.dt.uint32

    W128 = cfg.w128            # e.g. 20
    W16 = cfg.w16              # e.g. 160
    KC = cfg.kcap
    KCOLS = KC // 16           # compacted index columns per expert
    NJ = KC // 128             # 128-slot blocks per expert
    AW = W16                   # arr block width per expert
    NROWS = cfg.bcp + 1        # + trash row
    AluOp = mybir.AluOpType

    nc = bacc.Bacc("TRN2", target_bir_lowering=False, debug=debug,
                   num_devices=NCORES)
    nregs = [nc.alloc_register(mybir.EngineType.Pool, f"nf{e}") for e in range(E)]

    t_in = nc.declare_dram_parameter("t_in", [NROWS, CR], bf16, isOutput=False)
    attrs_in = nc.declare_dram_parameter("attrs", [16, W16 * E], fp32, isOutput=False)
    wt_in = nc.declare_dram_parameter("wt", [C, E * OUT], fp32, isOutput=False)
    biasb_in = nc.declare_dram_parameter("biasb", [128, E * 512], bf16, isOutput=False)
    iota_in = nc.declare_dram_parameter("iota_c", [16, W16 * E], fp32, isOutput=False)
    bp1_in = nc.declare_dram_parameter("bp1_c", [16, W16], fp32, isOutput=False)
    seq_in = nc.declare_dram_parameter("seq_c", [16, KCOLS], fp32, isOutput=False)
    out_ext = nc.declare_dram_parameter("out", [NROWS, CR], bf16, isOutput=True)
    if dump:
        idxw_dbg = nc.declare_dram_parameter("idxw_dbg", [16, W16], fp32, isOutput=True)
        comp_dbg = nc.declare_dram_parameter("comp_dbg", [16, E * KCOLS], fp32, isOutput=True)
        idx128_dbg = nc.declare_dram_parameter("idx128_dbg", [128, KCOLS], i16, isOutput=True)

    with tile.TileContext(nc) as tc:
        with (
            tc.tile_pool(name="const", bufs=1) as cpool,
            tc.tile_pool(name="route", bufs=6) as rpool,
            tc.tile_pool(name="xg", bufs=7) as xpool,
            tc.tile_pool(name="stg", bufs=5) as spool,
            tc.tile_pool(name="ps", bufs=6, space="PSUM") as ppool,
            tc.tile_pool(name="psn", bufs=2, space="PSUM") as pnpool,
        ):
            # ---- routing inputs first: they gate the whole pipeline ----
            attrs_sb = cpool.tile([16, W16 * E], fp32, tag="attrs")
            nc.sync.dma_start(out=attrs_sb[:], in_=attrs_in[:])
            iota_sb = cpool.tile([16, W16 * E], fp32)
            nc.scalar.dma_start(out=iota_sb[:], in_=iota_in[:])
            bp1_sb = cpool.tile([16, W16], fp32)
            nc.gpsimd.dma_start(out=bp1_sb[:], in_=bp1_in[:])

            # ---- weights / bias prep ----
            wt_f32 = cpool.tile([C, E * OUT], fp32)
            nc.sync.dma_start(out=wt_f32[:], in_=wt_in[:])
            wt_sb = cpool.tile([C, E * OUT], bf16)
            nc.vector.tensor_scalar_mul(wt_sb[:], wt_f32[:], ALPHA)

            biasb_sb = cpool.tile([128, E * 512], bf16)
            nc.sync.dma_start(out=biasb_sb[:], in_=biasb_in[:])

            ones16 = cpool.tile([1, 16], fp32)
            nc.vector.memset(ones16[:], 1.0)
            seqpos = cpool.tile([16, KCOLS], fp32)
            nc.gpsimd.dma_start(out=seqpos[:], in_=seq_in[:])
            neg1 = cpool.tile([16, KCOLS], fp32)
            nc.vector.memset(neg1[:], -1.0)

            a3 = attrs_sb[:].rearrange("p (w j) -> p w j", j=E)
            mx = cpool.tile([16, W16], fp32, tag="mx")
            nc.vector.tensor_reduce(out=mx[:], in_=a3, op=AluOp.max,
                                    axis=mybir.AxisListType.X)
            onehot = cpool.tile([16, W16 * E], fp32, tag="onehot")
            nc.vector.tensor_tensor(
                out=onehot[:].rearrange("p (w j) -> p w j", j=E),
                in0=a3,
                in1=mx[:, :, None].to_broadcast([16, W16, E]),
                op=AluOp.is_equal,
            )
            cand = cpool.tile([16, W16 * E], fp32, tag="cand")
            nc.vector.tensor_tensor(out=cand[:], in0=onehot[:], in1=iota_sb[:],
                                    op=AluOp.mult)
            idxw16 = cpool.tile([16, W16], fp32, tag="idxw16")
            nc.vector.tensor_reduce(out=idxw16[:],
                                    in_=cand[:].rearrange("p (w j) -> p w j", j=E),
                                    op=AluOp.min, axis=mybir.AxisListType.X)
            nc.vector.tensor_scalar_add(idxw16[:], idxw16[:], BIG)
            if dump:
                nc.sync.dma_start(out=idxw_dbg[:], in_=idxw16[:])

            # ---- per-expert candidate lists (arr = node_id or -1),
            # built expert-by-expert so sparse_gather can start early ----
            arr_all = cpool.tile([16, E * AW], fp32, tag="arr_all")
            for e in range(E):
                ae = arr_all[:, e * AW:(e + 1) * AW]
                nc.vector.tensor_scalar(ae, idxw16[:], float(e), None,
                                        op0=AluOp.is_equal)
                nc.vector.tensor_tensor(out=ae, in0=ae, in1=bp1_sb[:],
                                        op=AluOp.mult)
                nc.vector.tensor_scalar_add(ae, ae, -1.0)

            # ---- PE warm-up: HAM un-throttles after ~3.4us of sustained
            # matmul; run throwaway matmuls during the routing ramp ----
            for wb in range(10):
                warm_ps = ppool.tile([128, 512], fp32, tag="ps", space="PSUM")
                for wi in range(4):
                    nc.tensor.matmul(warm_ps[:, wi * 128:(wi + 1) * 128],
                                     lhsT=wt_sb[:, :128], rhs=wt_sb[:, :OUT],
                                     start=True, stop=True)

            # ---- per-expert pipeline ----
            sparse_insts = {}
            gather_insts = {}
            for e in range(E):
                compact = rpool.tile([16, KCOLS], fp32, tag="compact")
                nfound = rpool.tile([1, 1], u32, tag="nfound")
                sp = nc.gpsimd.sparse_gather(out=compact[:],
                                              in_=arr_all[:, e * AW:(e + 1) * AW],
                                              num_found=nfound[:])
                sparse_insts[e] = sp.ins
                nc.gpsimd.reg_load(nregs[e], nfound[:1, :1])
                if dump:
                    nc.sync.dma_start(out=comp_dbg[:, e * KCOLS:(e + 1) * KCOLS],
                                      in_=compact[:])

                # HW sparse_gather fills past num_found with garbage (CoreSim
                # pads -1); the Q7 gather/scatter trim trailing work by idx
                # value, so positions >= nf must be -1.
                nff = rpool.tile([1, 1], fp32, tag="nff")
                nc.scalar.copy(nff[:], nfound[:])
                nfps = pnpool.tile([16, 1], fp32, tag="nfps", space="PSUM")
                nc.tensor.matmul(nfps[:], lhsT=ones16[:], rhs=nff[:],
                                 start=True, stop=True)
                nf16 = rpool.tile([16, 1], fp32, tag="nf16")
                nc.scalar.copy(nf16[:], nfps[:])
                vmask = rpool.tile([16, KCOLS], mybir.dt.uint8, tag="vmask")
                nc.vector.tensor_scalar(vmask[:], seqpos[:], nf16[:, :1], None,
                                        op0=AluOp.is_lt)
                idxc = rpool.tile([16, KCOLS], fp32, tag="idxc")
                nc.vector.select(idxc[:], vmask[:], compact[:], neg1[:])
                idx16 = rpool.tile([16, KCOLS], i16, tag="idx16")
                nc.scalar.copy(idx16[:], idxc[:])
                idx128 = rpool.tile([128, KCOLS], i16, tag="idx128")
                for rep in range(8):
                    eng = nc.sync if rep % 2 == 0 else nc.scalar
                    eng.dma_start(out=idx128[rep * 16:(rep + 1) * 16, :],
                                  in_=idx16[:])
                if dump and e == 0:
                    nc.sync.dma_start(out=idx128_dbg[:], in_=idx128[:])

                # gather this expert's node blocks: [c=128, r, slot] bf16
                xg = xpool.tile([128, R, KC], bf16, tag="xg")
                gi = nc.gpsimd.dma_gather(
                    xg[:],
                    t_in[:, :],
                    idx128[:],
                    num_idxs=KC,
                    num_idxs_reg=nregs[e],
                    elem_size=CR,
                    transpose=True,
                )
                gather_insts[e] = gi.ins

                # matmuls: lhsT = X chunk [c, 128 slots], rhs = W[e]^T [c, out]
                stage = spool.tile([128, NJ, CR], bf16, tag="stage")
                for j in range(NJ):
                    for rg in range(R // 4):
                        ps = ppool.tile([128, 512], fp32, tag="ps", space="PSUM")
                        for ri in range(4):
                            r = rg * 4 + ri
                            nc.tensor.matmul(
                                ps[:, ri * 128:(ri + 1) * 128],
                                lhsT=xg[:, r, j * 128:(j + 1) * 128],
                                rhs=wt_sb[:, e * OUT:(e + 1) * OUT],
                                start=(ri == 0), stop=(ri == 3),
                            )
                        # psum + bias -> staging (Tile splits DVE/ACT)
                        dst = stage[:, j, rg * 512:(rg + 1) * 512]
                        nc.any.tensor_tensor(out=dst, in0=ps[:],
                                             in1=biasb_sb[:, e * 512:(e + 1) * 512],
                                             op=AluOp.add)

                # scatter rows back (out starts zeroed; one row per token)
                nc.gpsimd.dma_scatter_add(
                    out_ext[:, :],
                    stage[:],
                    idx128[:],
                    num_idxs=KC,
                    num_idxs_reg=nregs[e],
                    elem_size=CR,
                )

    nc.compile()
    return nc


# ---------------------------------------------------------------------------
# Host-side staging
# ---------------------------------------------------------------------------

def make_constants(cfg=FULL):
    iota_c = np.tile(np.arange(E, dtype=np.float32) - np.float32(BIG),
                     (16, cfg.w16, 1)).reshape(16, cfg.w16 * E)
    kcols = cfg.kcap // 16
    seq_c = (np.arange(cfg.kcap, dtype=np.float32).reshape(kcols, 16).T.copy())
    # node at [q, w'] is n = q*W16 + w'
    bp1_c = (np.arange(cfg.bcp, dtype=np.float32) + 1.0).reshape(16, cfg.w16)
    return iota_c, bp1_c, seq_c


def stage_core_inputs(Tc, attrs_c, weights, bias, cfg=FULL):
    """Build the device input map for one core from its f32 shard."""
    bc = Tc.shape[0]

    t_st = np.zeros((cfg.bcp + 1, CR), dtype=BF16)
    # node row layout: f = r*128 + c  (so gather-transpose lands c on partitions)
    t_st[:bc] = Tc.transpose(0, 2, 1).reshape(bc, CR).astype(BF16)

    at = np.zeros((cfg.bcp, E), np.float32)
    at[:bc] = attrs_c
    # [16, W16*E]: node q*W16 + w at [q, w*E:(w+1)*E]
    attrs_w = at.reshape(16, cfg.w16 * E)

    wt = np.ascontiguousarray(
        weights.transpose(2, 0, 1).reshape(C, E * OUT)).astype(np.float32)
    biasb = np.ascontiguousarray(np.broadcast_to(
        np.tile(bias[:, None, :], (1, 4, 1)).reshape(1, E * 512),
        (128, E * 512))).astype(BF16)

    iota_c, bp1_c, seq_c = make_constants(cfg)
    return {
        "t_in": t_st,
        "attrs": attrs_w,
        "wt": wt,
        "biasb": biasb,
        "iota_c": iota_c,
        "bp1_c": bp1_c,
        "seq_c": seq_c,
    }


def unstage_core_output(out_st, bc, cfg=FULL):
    """Device output [bcp+1, CR] bf16 (row = r-major) -> [bc, OUT, R] f32."""
    o = np.asarray(out_st[:bc], dtype=np.float32)
    return o.reshape(bc, R, OUT).transpose(0, 2, 1)


_NC_CACHE = {}


def _get_nc(cfg=FULL):
    key = (cfg.bcp, cfg.kcap)
    if key not in _NC_CACHE:
        _NC_CACHE[key] = build_nc(cfg)
    return _NC_CACHE[key]


def kernel(T, node_attrs, weights, bias):
    T = np.asarray(T, dtype=np.float32)
    node_attrs = np.asarray(node_attrs, dtype=np.float32)
    weights = np.asarray(weights, dtype=np.float32)
    bias = np.asarray(bias, dtype=np.float32)

    cfg = FULL
    bc = B // NCORES
    in_maps = []
    for c in range(NCORES):
        sl = slice(c * bc, (c + 1) * bc)
        in_maps.append(stage_core_inputs(T[sl], node_attrs[sl], weights, bias, cfg))

    nc = _get_nc(cfg)
    res = run_bass_kernel_spmd(nc, in_maps, core_ids=list(range(NCORES)))

    outs = [unstage_core_output(res.results[c]["out"], bc, cfg)
            for c in range(NCORES)]
    return np.concatenate(outs, axis=0).reshape(B, OUT, R)
